# revision 30
# baseline (speedup 1.0000x reference)
"""Distributed Trainium2 kernel for a single causal attention head (v3).

Module: k,q,v = x@W{k,q,v}.T ; a = softmax(causal(q@k.T/sqrt(64))) ; out = a@v
Shapes: x (4, 4096, 1024) f32; W* (64, 1024) f32; out (4, 4096, 64) f32.

Sharding (one SPMD launch, 8 cores, no collectives): 4 batches x 2
key-parity halves. Core c: batch b=c//2, parity p=c%2. The 32 key chunks
(128 tokens) of a batch are split by parity, which makes the causal work
and the instruction structure identical on every core. The host hands
each core x[b].T with token columns permuted so the core's own-parity
key blocks sit at even 128-block positions.

v3 structure: projections and attention are interleaved per 1024-token
region r, and ALL key-chunk pairs flow through one global depth-2
software pipeline (emit S(m); AV(m-2); exp(m)). Depth 2 keeps the
in-order PE from blocking the next pair's S behind an AV that waits on
ACT, so the exp stream runs back-to-back; depth 1 put exp's 1.1us on a
serial S->exp->AV->S loop (measured 1.42us/pair). Pipes drain lazily
into the following stage. Diagonal pairs compute only 768 of 1024 score
columns (the second chunk of the pair is dead for query offsets <256).

Per core: K^T,V^T (packed [Wk|Wv]) for own-parity tokens, Q^T ([Wq|0])
for ALL tokens; V^T -> V by PE transpose with a ones column so softmax
sums ride along row 64 of the AV output; S^T = K^T.T @ Q^T (contraction
over the full 128 partitions with Q's V-rows zeroed), P^T = exp(S^T/8)
on ACT, O'^T(65,512) += [V|1].T @ P^T. The partial [O'^T; l] (65, 4096)
goes to DRAM; the host adds the two parity partials per batch, divides
by the summed denominators, and un-permutes.

Compute dtype: bf16 matmul operands with f32 PSUM accumulation
(~3e-3 rel err).
"""

import numpy as np

B, T, E, H = 4, 4096, 1024, 64
P = 128           # partitions
QC = 512          # query chunk (matmul moving free dim)
KC = 128          # key chunk
DC = 768          # trimmed diagonal-pair score columns (512 + 256)
ETILES = E // P   # 8 contraction tiles
NREG = 4          # 1024-column load/projection regions
NQCH = T // QC    # 8 query chunks
RC = T // NREG    # 1024 columns per region

_CACHE = {}


def _build_graph():
    import concourse.bass as bass
    import concourse.tile as tile
    from concourse import bacc, mybir
    f32 = mybir.dt.float32
    bf16 = mybir.dt.bfloat16
    AF = mybir.ActivationFunctionType
    ALU = mybir.AluOpType

    nc = bacc.Bacc("TRN2", target_bir_lowering=False, debug=False, num_devices=8)
    xTa_d = nc.dram_tensor("xTa", [E, T], bf16, kind="ExternalInput").ap()
    wkv_d = nc.dram_tensor("wkv", [E, P], bf16, kind="ExternalInput").ap()
    wq_d = nc.dram_tensor("wq", [E, P], bf16, kind="ExternalInput").ap()
    dmask_d = nc.dram_tensor("dmask", [P, DC], bf16, kind="ExternalInput").ap()
    ident_d = nc.dram_tensor("ident", [P, P], bf16, kind="ExternalInput").ap()
    out_d = nc.dram_tensor("o", [H + 1, NQCH, QC], f32, kind="ExternalOutput").ap()

    with tile.TileContext(nc) as tc:
        with (
            tc.tile_pool(name="consts", bufs=1) as consts,
            tc.tile_pool(name="xin", bufs=1) as xin,
            tc.tile_pool(name="keep", bufs=1) as keep,
            tc.tile_pool(name="work", bufs=3) as work,
            tc.tile_pool(name="psum", bufs=1, space="PSUM") as psum,
        ):
            # ---- constants: wq gates the first matmul chain, goes first ----
            wq_sb = consts.tile([P, ETILES, P], bf16)
            nc.sync.dma_start(wq_sb[:], wq_d.rearrange("(ko p) m -> p ko m", p=P))
            wkv_sb = consts.tile([P, ETILES, P], bf16)
            nc.sync.dma_start(wkv_sb[:], wkv_d.rearrange("(ko p) m -> p ko m", p=P))

            # ---- x region loads from the idle GpSimd sequencer; the two
            # small consts go after region 0 so they don't steal its BW ----
            xt = []
            dmask_sb = consts.tile([P, DC], bf16)
            ident = consts.tile([P, P], bf16)
            for r in range(NREG):
                xr = xin.tile([P, ETILES, RC], bf16, tag=f"xt{r}", name=f"xt{r}")
                for ko in range(ETILES):
                    nc.gpsimd.dma_start(
                        xr[:, ko], xTa_d[ko * P:(ko + 1) * P, r * RC:(r + 1) * RC])
                xt.append(xr)
                if r == 0:
                    nc.gpsimd.dma_start(dmask_sb[:], dmask_d[:])
                    nc.gpsimd.dma_start(ident[:], ident_d[:])

            kv = [keep.tile([P, QC], bf16, tag=f"kv{r}", name=f"kv{r}")
                  for r in range(NREG)]
            vv = [keep.tile([P, 4, H + 1], bf16, tag=f"v{r}", name=f"v{r}")
                  for r in range(NREG)]
            qq = [keep.tile([P, QC], bf16, tag=f"q{j}", name=f"q{j}")
                  for j in range(NQCH)]

            def qproj(r, half):
                pq = psum.tile([P, QC], f32, tag="proj", bufs=2,
                               name=f"pq_{r}_{half}")
                for ko in range(ETILES):
                    nc.tensor.matmul(pq[:], wq_sb[:, ko],
                                     xt[r][:, ko, half * QC:(half + 1) * QC],
                                     start=(ko == 0), stop=(ko == ETILES - 1))
                nc.vector.tensor_copy(qq[2 * r + half][:], pq[:])

            def kvproj(r):
                # ones column for the denominator row of the AV output
                nc.vector.memset(vv[r][:], 1.0)
                # The PE crashes on strided moving operands, so compact the
                # even (own-parity) 128-blocks into a contiguous tile on DVE.
                xkv = work.tile([P, ETILES, QC], bf16, tag="xkv", bufs=2)
                for ko in range(ETILES):
                    nc.vector.tensor_copy(
                        xkv[:, ko],
                        xt[r][:, ko].rearrange("p (u v c) -> p u v c",
                                               v=2, c=KC)[:, :, 0, :])
                pkv = psum.tile([P, QC], f32, tag="proj", bufs=2,
                                name=f"pkv_{r}")
                for ko in range(ETILES):
                    nc.tensor.matmul(pkv[:], wkv_sb[:, ko], xkv[:, ko],
                                     start=(ko == 0), stop=(ko == ETILES - 1))
                nc.vector.tensor_copy(kv[r][:], pkv[:])
                # V^T -> V for the region's 4 local key chunks
                for i in range(4):
                    ptr = psum.tile([P, P], bf16, tag="proj", bufs=2,
                                    name=f"ptr_{r}_{i}")
                    nc.tensor.transpose(ptr[:], kv[r][:, i * KC:(i + 1) * KC],
                                        ident[:])
                    nc.vector.tensor_copy(vv[r][:, i, 0:H], ptr[:, H:P])

            class Pipe:
                """Per-query-chunk output accumulator state."""

                def __init__(self, j):
                    self.j = j
                    self.po = psum.tile([H + 1, QC], f32, tag="po", bufs=2,
                                        name=f"po_{j}")
                    self.first = True

            pend = []  # global depth-2 pipeline of (pipe, m, pt, masked, last)

            def av(pipe, m, pt, masked, last):
                po, v = pipe.po, vv[m // 2]
                s0 = 2 * (m % 2)
                if masked:
                    # slot 1 only covers query offsets 256:512 (rest is dead)
                    if pipe.first:  # diag first: full-width u0 carries start
                        nc.tensor.matmul(po[:], v[:, s0, :], pt[:, 0:QC],
                                         start=True, stop=False)
                        nc.tensor.matmul(po[:, QC // 2:QC], v[:, s0 + 1, :],
                                         pt[:, QC:DC], start=False, stop=last)
                    else:           # diag last: full-width u0 carries stop
                        nc.tensor.matmul(po[:, QC // 2:QC], v[:, s0 + 1, :],
                                         pt[:, QC:DC], start=False, stop=False)
                        nc.tensor.matmul(po[:], v[:, s0, :], pt[:, 0:QC],
                                         start=False, stop=last)
                else:
                    for u in range(2):
                        nc.tensor.matmul(po[:], v[:, s0 + u, :],
                                         pt[:, u * QC:(u + 1) * QC],
                                         start=(pipe.first and u == 0),
                                         stop=(last and u == 1))
                pipe.first = False
                if last:
                    ost = work.tile([H + 1, QC], f32, tag="ost", bufs=2,
                                    name=f"ost_{pipe.j}")
                    nc.vector.tensor_copy(ost[:], po[:])
                    nc.sync.dma_start(out_d[:, pipe.j], ost[:])

            def pop_av():
                if len(pend) >= 2:
                    av(*pend.pop(0))

            def feed(pipe, m, masked=False, last=False):
                j = pipe.j
                off = (m % 2) * 2 * KC
                cols = DC if masked else 2 * QC
                ps = psum.tile([P, cols], f32, tag="ps", bufs=2,
                               name=f"ps_{j}_{m}")
                nc.tensor.matmul(ps[:, 0:QC], kv[m // 2][:, off:off + KC],
                                 qq[j][:], start=True, stop=True)
                if masked:
                    nc.tensor.matmul(ps[:, QC:DC],
                                     kv[m // 2][:, off + KC:off + 2 * KC],
                                     qq[j][:, QC // 2:QC],
                                     start=True, stop=True)
                else:
                    nc.tensor.matmul(ps[:, QC:2 * QC],
                                     kv[m // 2][:, off + KC:off + 2 * KC],
                                     qq[j][:], start=True, stop=True)
                pop_av()
                pt = work.tile([P, cols], bf16, tag="pt", bufs=4,
                               name=f"pt_{j}_{m}")
                nc.scalar.activation(pt[:], ps[:], AF.Exp,
                                     scale=float(H) ** -0.5)
                if masked:
                    nc.vector.tensor_tensor(pt[:], pt[:], dmask_sb[:],
                                            ALU.mult)
                pend.append((pipe, m, pt, masked, last))

            for r in range(NREG):
                j_hi, j_lo = 2 * r + 1, 2 * r
                qproj(r, 1)
                hi = Pipe(j_hi)
                for m in range(0, j_hi - 1):   # pairs on prior regions' keys
                    feed(hi, m)
                qproj(r, 0)
                kvproj(r)
                feed(hi, j_hi - 1)
                feed(hi, j_hi, masked=True, last=True)
                lo = Pipe(j_lo)
                feed(lo, j_lo, masked=True, last=(j_lo == 0))
                for m in range(0, j_lo):
                    feed(lo, m, last=(m == j_lo - 1))
            while pend:
                av(*pend.pop(0))

    nc.compile()
    return nc


def _get_graph():
    if "g" not in _CACHE:
        _CACHE["g"] = _build_graph()
    return _CACHE["g"]


def _perm(p: int) -> np.ndarray:
    """Token column permutation for parity p: own-parity 128-blocks at even
    block positions (identity for p=0, adjacent-block swap for p=1)."""
    blocks = np.arange(T // KC).reshape(-1, 2)
    if p == 1:
        blocks = blocks[:, ::-1]
    return (blocks.reshape(-1)[:, None] * KC + np.arange(KC)[None, :]).reshape(-1)


def _make_masks(p: int) -> np.ndarray:
    """Diagonal-pair mask in permuted column space, trimmed layout
    [slot0 (512 cols) | slot1 at query offsets 256:512 (256 cols)]."""
    perm = _perm(p)
    sigma = perm[:QC] % QC  # within-chunk token offset pattern (j-independent)
    s = np.arange(P)[:, None]
    m = np.empty((P, DC), np.float32)
    m[:, 0:QC] = (sigma[None, :] - s - KC * p) >= 0
    m[:, QC:DC] = (sigma[None, QC // 2:QC] - s - KC * (p + 2)) >= 0
    return m


def _run(x, Wk, Wq, Wv, trace=False):
    from concourse.bass_utils import run_bass_kernel_spmd
    import ml_dtypes

    x = np.asarray(x, dtype=np.float32)
    Wk = np.asarray(Wk, dtype=np.float32)
    Wq = np.asarray(Wq, dtype=np.float32)
    Wv = np.asarray(Wv, dtype=np.float32)

    conv = lambda a: np.asarray(a, dtype=ml_dtypes.bfloat16)
    wkv = conv(np.concatenate([Wk.T, Wv.T], axis=1))
    wq = conv(np.concatenate([Wq.T, np.zeros((E, H), np.float32)], axis=1))
    masks = [conv(_make_masks(0)), conv(_make_masks(1))]
    ident_np = conv(np.eye(P, dtype=np.float32))
    perms = [_perm(0), _perm(1)]

    in_maps = []
    xTb = {}
    for c in range(8):
        b, p = c // 2, c % 2
        if (b, p) not in xTb:
            xTb[(b, p)] = conv(x[b].T[:, perms[p]])
        in_maps.append({"xTa": xTb[(b, p)], "wkv": wkv, "wq": wq,
                        "dmask": masks[p], "ident": ident_np})

    nc = _get_graph()
    res = run_bass_kernel_spmd(nc, in_maps, core_ids=list(range(8)), trace=trace)

    out = np.empty((B, T, H), dtype=np.float32)
    for b in range(B):
        o0 = res.results[2 * b]["o"].reshape(H + 1, T)
        o1 = res.results[2 * b + 1]["o"].reshape(H + 1, T)
        # p=1 columns are block-swapped; un-permute before merging
        o1 = o1[:, perms[1]]
        s = o0 + o1
        out[b] = (s[0:H] / s[H:H + 1]).T
    return out, res.exec_time_ns


def kernel(x, Wk, Wq, Wv):
    out, _ = _run(x, Wk, Wq, Wv)
    return out
